# revision 28
# baseline (speedup 1.0000x reference)
"""Trainium2 Bass kernel for nn_ATPModule_38062000177838 (topk_masking).

The only heavy compute in the module is the pair of mean-reductions over
attention_weights[:, :, :576, :576] (S_self) and [:, :, 576:, :576]
(S_cross).  hidden_states / position_ids pass through unchanged, and the
pooling + tiny MLP + sigmoid masks are O(B*576) host-side work.

Sharding: 64 (batch, head) pairs -> 8 per core (cores 0-3: batch 0,
cores 4-7: batch 1).  Each core reduces its 8 pairs to partial column
sums (row 0: self rows 0:576, row 1: cross rows 576:1024); the host
combines partials, divides by the counts, and runs the tiny MLP + masks
in numpy float32.

Device kernel design (per core):
- The host packs each pair's [1024, 576] slice to bf16 (halves HBM
  traffic; the 18k-element mean washes out quantization noise, measured
  mask error ~1e-3) in a layout where SBUF partition p holds rows
  8p..8p+7 -- each partition's tile line is one contiguous 9KB DRAM run,
  so the DMA sustains ~fabric-rate.
- Self rows (0:576) are exactly partitions 0:72, so a single [128, 2]
  0/1 weight matrix routes self/cross sums into PSUM rows 0/1 via
  TensorEngine matmuls (bf16, 1 cycle/column), 9 N=512 matmuls per pair
  (the eight 64-col chunk tails are covered by one strided matmul into a
  second PSUM bank and summed on host).
- All aw DMAs go on the strict-FIFO sync HWDGE ring in consumption
  order; the first/last pair loads are split so the PE starts ~1us after
  the first piece lands and trails the stream tail.  A few dummy matmuls
  during the DMA lead-in warm the PE out of its cold HAM clock state.
"""

import ml_dtypes
import numpy as np

import concourse.bacc as bacc
import concourse.mybir as mybir
import concourse.tile as tile
from concourse.bass_utils import run_bass_kernel_spmd

P = 128          # SBUF partitions
LV = 576         # num vision tokens
S = 1024         # sequence length
NPAIRS = 8       # (batch, head) pairs per core
NCHUNK = 8       # 1024 rows / 128 partitions
N_CORES = 8
N_HEADS = 32
B = 2

_NC_CACHE = None

# tuned on hardware; see bench.py
BUFS = 6
FIRST_SPLIT = ((0, 2), (2, 8))
LAST_SPLIT = ((0, 4), (4, 8))
WARM_MMS = 2
GROUP = 1


def build_nc(bufs=None, first_split=None, last_split=None, warm_mms=None,
             group=None, wsb_warm=False):
    bufs = BUFS if bufs is None else bufs
    first_split = FIRST_SPLIT if first_split is None else first_split
    last_split = LAST_SPLIT if last_split is None else last_split
    warm_mms = WARM_MMS if warm_mms is None else warm_mms
    group = GROUP if group is None else group
    f32 = mybir.dt.float32
    nc = bacc.Bacc("TRN2", target_bir_lowering=False)
    bf16 = mybir.dt.bfloat16
    # aw is host-packed to [pairs, S, LV] bf16: halves the HBM traffic
    # (the 18k-element mean reduction washes out bf16 quantization noise)
    # and the kernel reads contiguous 9KB-per-partition runs.
    aw = nc.dram_tensor("aw", [NPAIRS, S, LV], bf16, kind="ExternalInput")
    wts = nc.dram_tensor("wts", [P, 2], bf16, kind="ExternalInput")
    out = nc.dram_tensor("out", [2, 1024], f32, kind="ExternalOutput")

    with tile.TileContext(nc) as tc:
        with tc.tile_pool(name="io", bufs=bufs) as pool, \
             tc.tile_pool(name="consts", bufs=1) as cpool, \
             tc.tile_pool(name="psum", bufs=1, space="PSUM") as ppool:
            w_sb = cpool.tile([P, 2], bf16, tag="wsb")
            nc.scalar.dma_start(w_sb[:, :], wts[:, :])
            # Layout: partition p holds rows 8p..8p+7 (chunk n = row 8p+n).
            # Self rows (0:576) are exactly partitions 0:72, so one weight
            # matrix [self_mask, cross_mask] serves every matmul.
            w2 = w_sb[:, 0:2]

            # ps_a row 0/1 = self/cross sums for columns 0:512.
            # ps_b accumulates the eight 64-wide column tails (cols 512:576
            # of each chunk) side by side; the host sums the 8 groups.
            ps_a = ppool.tile([2, 512], f32, tag="psa")
            ps_b = ppool.tile([2, 512], f32, tag="psb")

            # Dummy matmuls fill the PE's cold HAM window (~3.4us at
            # 1.2GHz) during the DMA lead-in so real matmuls run at
            # 2.4GHz; also consumes the weights-DMA wait.
            ps_w = ppool.tile([2, 512], f32, tag="psw")
            dummy = cpool.tile([P, 512], bf16, tag="dummy")
            nc.vector.memset(dummy[:, :], 1.0)
            for _ in range(warm_mms):
                nc.tensor.matmul(ps_w[:, :], dummy[:, 0:2], dummy[:, :],
                                 start=True, stop=True)
            if wsb_warm:
                # Consume the weights-DMA wait before the first real matmul
                # (which may then carry only the tile-DMA wait).
                nc.tensor.matmul(ps_w[:, :], w_sb[:, 0:2], dummy[:, :],
                                 start=True, stop=True)

            # Full reduction on the TensorEngine in bf16 (1 cycle/column),
            # accumulating into f32 PSUM across all pairs.  `group` pairs
            # share one DMA (fewer, larger transfers).
            for g0 in range(0, NPAIRS, group):
                t = pool.tile([P, group, NCHUNK, LV], bf16, tag="awt")
                src = aw[g0:g0 + group].rearrange("q (p n) m -> p q n m",
                                                  n=NCHUNK)
                first = g0 == 0
                lastg = g0 + group == NPAIRS
                if first and group == 1 and first_split:
                    # Split the first load so the PE starts ~1us after the
                    # first piece lands instead of waiting for the full
                    # tile.  All aw DMAs stay on the sync ring: it is
                    # strict-FIFO, which keeps completion order equal to PE
                    # consumption order.
                    for c0, c1 in first_split:
                        nc.sync.dma_start(t[:, 0, c0:c1, :],
                                          src[:, 0, c0:c1, :])
                elif lastg and group == 1 and last_split:
                    # Split the last load too: the PE then trails the tail
                    # of the stream chunk by chunk instead of waiting for
                    # the whole tile, pulling the final matmul earlier.
                    for c0, c1 in last_split:
                        nc.sync.dma_start(t[:, 0, c0:c1, :],
                                          src[:, 0, c0:c1, :])
                else:
                    nc.sync.dma_start(t[:, :, :, :], src)
                for q in range(group):
                    st = first and q == 0
                    lastp = lastg and q == group - 1
                    # One strided matmul covers all eight 64-col chunk
                    # tails of this pair.
                    nc.tensor.matmul(ps_b[:, :], w2, t[:, q, :, 512:LV],
                                     start=st, stop=lastp)
                    for n in range(NCHUNK):
                        nc.tensor.matmul(ps_a[:, :], w2, t[:, q, n, 0:512],
                                         start=st and n == 0,
                                         stop=lastp and n == NCHUNK - 1)

            out_sb = cpool.tile([2, 1024], f32, tag="outsb")
            # ps_b finished one matmul earlier, so its copy (on the scalar
            # engine) overlaps ps_a's final matmuls and the ps_a copy.
            nc.scalar.copy(out_sb[:, 512:1024], ps_b[:, :])
            nc.vector.tensor_copy(out_sb[:, 0:512], ps_a[:, :])
            nc.sync.dma_start(out[:, :], out_sb[:, :])
    nc.compile()
    return nc


def make_weights():
    wts = np.zeros((P, 2), ml_dtypes.bfloat16)
    wts[:72, 0] = 1.0  # partitions 0:72 = rows 0:576 -> self row
    wts[72:, 1] = 1.0  # partitions 72:128 = rows 576:1024 -> cross row
    return wts


def device_partial_sums(aw, trace=False):
    """Run the 8-core SPMD reduction. aw: [B, H, S, S] f32.

    Returns (S_self_sum, S_cross_sum) each [B, LV] (unnormalized column
    sums over heads x rows), plus the BassKernelResults."""
    global _NC_CACHE
    if _NC_CACHE is None:
        _NC_CACHE = build_nc()
    nc = _NC_CACHE
    wts = make_weights()
    in_maps = []
    for c in range(N_CORES):
        b, h0 = divmod(c, 4)
        in_maps.append(
            {"aw": aw[b, h0 * 8:h0 * 8 + 8, :, :LV].astype(ml_dtypes.bfloat16),
             "wts": wts})
    res = run_bass_kernel_spmd(nc, in_maps, core_ids=list(range(N_CORES)),
                               trace=trace)
    parts = np.stack([res.results[c]["out"] for c in range(N_CORES)])
    # cols 0:512 directly; cols 512:1024 hold the eight 64-wide chunk
    # tails side by side -> sum the 8 groups to get cols 512:576.
    def unpack(rows):  # rows: [4, 1024] for one batch/score row
        head = rows[:, 0:512].sum(axis=0)
        tail = rows[:, 512:1024].reshape(-1, 8, 64).sum(axis=(0, 1))
        return np.concatenate([head, tail])
    self_sum = np.stack([unpack(parts[4 * b:4 * b + 4, 0])
                         for b in range(B)]).astype(np.float32)
    cross_sum = np.stack([unpack(parts[4 * b:4 * b + 4, 1])
                          for b in range(B)]).astype(np.float32)
    return self_sum, cross_sum, res


def _sigmoid(x):
    x = np.asarray(x, np.float32)
    out = np.empty_like(x)
    pos = x >= 0
    out[pos] = 1.0 / (1.0 + np.exp(-x[pos]))
    ex = np.exp(x[~pos])
    out[~pos] = ex / (1.0 + ex)
    return out


def _adaptive_pool(x, out_size):
    # matches torch.nn.AdaptiveAvgPool1d over the last axis
    L = x.shape[-1]
    i = np.arange(out_size)
    starts = (i * L) // out_size
    ends = ((i + 1) * L + out_size - 1) // out_size
    zeros = np.zeros(x.shape[:-1] + (1,), x.dtype)
    cs = np.concatenate([zeros, np.cumsum(x, axis=-1, dtype=np.float32)],
                        axis=-1)
    lengths = (ends - starts).astype(np.float32)
    return ((cs[..., ends] - cs[..., starts]) / lengths).astype(np.float32)


def postprocess(S_self, S_cross, W1, b1, Wr, br, Ws, bs, lv=LV):
    LAMBDA_SAMPLE = np.float32(3.0)
    TEMPERATURE = np.float32(100.0)
    Bn = S_self.shape[0]

    S_red = ((S_self + S_cross) * np.float32(0.5)).astype(np.float32)

    grid = int(lv ** 0.5)
    stride = 2
    num_sampled = (grid // stride) ** 2
    R_s = num_sampled / lv
    ii, jj = np.meshgrid(np.arange(0, grid, stride),
                         np.arange(0, grid, stride), indexing="ij")
    sampled_idx = (ii * grid + jj).reshape(-1)
    smask = np.zeros((lv,), bool)
    smask[sampled_idx] = True
    S_spat = np.where(smask[None, :],
                      np.float32(1.0 - R_s * LAMBDA_SAMPLE),
                      np.float32(-100.0)).astype(np.float32)
    S_spat = np.broadcast_to(S_spat, (Bn, lv))

    pooled = np.concatenate([_adaptive_pool(S_self, 256),
                             _adaptive_pool(S_cross, 256)], axis=-1)
    shared = np.maximum(pooled @ W1 + b1, np.float32(0.0)).astype(np.float32)
    theta_r = _sigmoid(shared @ Wr + br)
    theta_s = _sigmoid(shared @ Ws + bs)

    mask_r = _sigmoid((S_red - theta_r) * TEMPERATURE)
    mask_s = _sigmoid((S_spat - theta_s) * TEMPERATURE)
    mask = np.maximum(mask_r, mask_s)
    return mask_r, mask_s, mask


def kernel(hidden_states, attention_weights, position_ids, num_vision_tokens,
           W1, b1, Wr, br, Ws, bs):
    hs = np.asarray(hidden_states)
    aw = np.asarray(attention_weights, np.float32)
    pos = np.asarray(position_ids)
    Lv = int(num_vision_tokens)
    W1 = np.asarray(W1, np.float32)
    b1 = np.asarray(b1, np.float32)
    Wr = np.asarray(Wr, np.float32)
    br = np.asarray(br, np.float32)
    Ws = np.asarray(Ws, np.float32)
    bs = np.asarray(bs, np.float32)

    if Lv == LV and aw.shape == (B, N_HEADS, S, S):
        self_sum, cross_sum, _ = device_partial_sums(aw)
        S_self = (self_sum / np.float32(N_HEADS * LV)).astype(np.float32)
        S_cross = (cross_sum / np.float32(N_HEADS * (S - LV))).astype(
            np.float32)
    else:
        # generic fallback for unexpected shapes: full reduction on host
        S_self = aw[:, :, :Lv, :Lv].mean(axis=(1, 2), dtype=np.float32)
        S_cross = aw[:, :, Lv:, :Lv].mean(axis=(1, 2), dtype=np.float32)

    mask_r, mask_s, mask = postprocess(S_self, S_cross, W1, b1, Wr, br,
                                       Ws, bs, lv=Lv)
    return hs, pos, mask_r, mask_s, mask


# revision 30
# speedup vs baseline: 1.0700x; 1.0700x over previous
"""Trainium2 Bass kernel for nn_ATPModule_38062000177838 (topk_masking).

The only heavy compute in the module is the pair of mean-reductions over
attention_weights[:, :, :576, :576] (S_self) and [:, :, 576:, :576]
(S_cross).  hidden_states / position_ids pass through unchanged, and the
pooling + tiny MLP + sigmoid masks are O(B*576) host-side work.

Sharding: 64 (batch, head) pairs -> 8 per core (cores 0-3: batch 0,
cores 4-7: batch 1).  Each core reduces its 8 pairs to partial column
sums (row 0: self rows 0:576, row 1: cross rows 576:1024); the host
combines partials, divides by the counts, and runs the tiny MLP + masks
in numpy float32.

Device kernel design (per core):
- The host packs each pair's [1024, 576] slice to bf16 (halves HBM
  traffic; the 18k-element mean washes out quantization noise, measured
  mask error ~1e-3) in a layout where SBUF partition p holds rows
  8p..8p+7 -- each partition's tile line is one contiguous 9KB DRAM run,
  so the DMA sustains ~fabric-rate.
- Self rows (0:576) are exactly partitions 0:72, so a single [128, 2]
  0/1 weight matrix routes self/cross sums into PSUM rows 0/1 via
  TensorEngine matmuls (bf16, 1 cycle/column), 9 N=512 matmuls per pair
  (the eight 64-col chunk tails are covered by one strided matmul into a
  second PSUM bank and summed on host).
- All aw DMAs go on the strict-FIFO sync HWDGE ring in consumption
  order; the first/last pair loads are split so the PE starts ~1us after
  the first piece lands and trails the stream tail.  A few dummy matmuls
  during the DMA lead-in warm the PE out of its cold HAM clock state.
"""

import ml_dtypes
import numpy as np

import concourse.bacc as bacc
import concourse.mybir as mybir
import concourse.tile as tile
from concourse.bass_utils import run_bass_kernel_spmd

P = 128          # SBUF partitions
LV = 576         # num vision tokens
S = 1024         # sequence length
NPAIRS = 8       # (batch, head) pairs per core
NCHUNK = 8       # 1024 rows / 128 partitions
N_CORES = 8
N_HEADS = 32
B = 2

_NC_CACHE = None

# tuned on hardware; see bench.py
BUFS = 6
FIRST_SPLIT = ((0, 2), (2, 8))
LAST_SPLIT = ((0, 4), (4, 6), (6, 8))
WARM_MMS = 2
GROUP = 1


def _make_bacc(skip_const_memsets):
    """Construct the Bacc assembler.  With skip_const_memsets, suppress the
    four const-AP SBUF memsets Bass.__init__ emits on GpSimd: nothing in
    this kernel reads the const APs, and they sit on the preamble critical
    path before the first DMA trigger."""
    if not skip_const_memsets:
        return bacc.Bacc("TRN2", target_bir_lowering=False)
    import concourse.bass as bass
    cls = bass.BassGpSimd
    orig = cls.memset

    def memset_patch(self, ap, constant):
        name = getattr(getattr(ap, "tensor", None), "name", "")
        if isinstance(name, str) and name.startswith("const-"):
            return None
        return orig(self, ap, constant)

    cls.memset = memset_patch
    try:
        return bacc.Bacc("TRN2", target_bir_lowering=False)
    finally:
        cls.memset = orig


def build_nc(bufs=None, first_split=None, last_split=None, warm_mms=None,
             group=None, wsb_warm=False, skip_const=True):
    bufs = BUFS if bufs is None else bufs
    first_split = FIRST_SPLIT if first_split is None else first_split
    last_split = LAST_SPLIT if last_split is None else last_split
    warm_mms = WARM_MMS if warm_mms is None else warm_mms
    group = GROUP if group is None else group
    f32 = mybir.dt.float32
    nc = _make_bacc(skip_const)
    bf16 = mybir.dt.bfloat16
    # aw is host-packed to [pairs, S, LV] bf16: halves the HBM traffic
    # (the 18k-element mean reduction washes out bf16 quantization noise)
    # and the kernel reads contiguous 9KB-per-partition runs.
    aw = nc.dram_tensor("aw", [NPAIRS, S, LV], bf16, kind="ExternalInput")
    wts = nc.dram_tensor("wts", [P, 2], bf16, kind="ExternalInput")
    out = nc.dram_tensor("out", [2, 1024], f32, kind="ExternalOutput")

    with tile.TileContext(nc) as tc:
        with tc.tile_pool(name="io", bufs=bufs) as pool, \
             tc.tile_pool(name="consts", bufs=1) as cpool, \
             tc.tile_pool(name="psum", bufs=1, space="PSUM") as ppool:
            w_sb = cpool.tile([P, 2], bf16, tag="wsb")
            nc.scalar.dma_start(w_sb[:, :], wts[:, :])
            # Layout: partition p holds rows 8p..8p+7 (chunk n = row 8p+n).
            # Self rows (0:576) are exactly partitions 0:72, so one weight
            # matrix [self_mask, cross_mask] serves every matmul.
            w2 = w_sb[:, 0:2]

            # ps_a row 0/1 = self/cross sums for columns 0:512.
            # ps_b accumulates the eight 64-wide column tails (cols 512:576
            # of each chunk) side by side; the host sums the 8 groups.
            ps_a = ppool.tile([2, 512], f32, tag="psa")
            ps_b = ppool.tile([2, 512], f32, tag="psb")

            # Dummy matmuls fill the PE's cold HAM window (~3.4us at
            # 1.2GHz) during the DMA lead-in so real matmuls run at
            # 2.4GHz; also consumes the weights-DMA wait.
            ps_w = ppool.tile([2, 512], f32, tag="psw")
            dummy = cpool.tile([P, 512], bf16, tag="dummy")
            nc.vector.memset(dummy[:, :], 1.0)
            for _ in range(warm_mms):
                nc.tensor.matmul(ps_w[:, :], dummy[:, 0:2], dummy[:, :],
                                 start=True, stop=True)
            if wsb_warm:
                # Consume the weights-DMA wait before the first real matmul
                # (which may then carry only the tile-DMA wait).
                nc.tensor.matmul(ps_w[:, :], w_sb[:, 0:2], dummy[:, :],
                                 start=True, stop=True)

            # Full reduction on the TensorEngine in bf16 (1 cycle/column),
            # accumulating into f32 PSUM across all pairs.  `group` pairs
            # share one DMA (fewer, larger transfers).
            for g0 in range(0, NPAIRS, group):
                t = pool.tile([P, group, NCHUNK, LV], bf16, tag="awt")
                src = aw[g0:g0 + group].rearrange("q (p n) m -> p q n m",
                                                  n=NCHUNK)
                first = g0 == 0
                lastg = g0 + group == NPAIRS
                if first and group == 1 and first_split:
                    # Split the first load so the PE starts ~1us after the
                    # first piece lands instead of waiting for the full
                    # tile.  All aw DMAs stay on the sync ring: it is
                    # strict-FIFO, which keeps completion order equal to PE
                    # consumption order.
                    for c0, c1 in first_split:
                        nc.sync.dma_start(t[:, 0, c0:c1, :],
                                          src[:, 0, c0:c1, :])
                elif lastg and group == 1 and last_split:
                    # Split the last load too: the PE then trails the tail
                    # of the stream chunk by chunk instead of waiting for
                    # the whole tile, pulling the final matmul earlier.
                    for c0, c1 in last_split:
                        nc.sync.dma_start(t[:, 0, c0:c1, :],
                                          src[:, 0, c0:c1, :])
                else:
                    nc.sync.dma_start(t[:, :, :, :], src)
                for q in range(group):
                    st = first and q == 0
                    lastp = lastg and q == group - 1
                    for n in range(NCHUNK):
                        nc.tensor.matmul(ps_a[:, :], w2, t[:, q, n, 0:512],
                                         start=st and n == 0,
                                         stop=lastp and n == NCHUNK - 1)
                    # One strided matmul covers all eight 64-col chunk
                    # tails.  It reads the whole tile, so it runs LAST:
                    # putting it first would queue the final pair's entire
                    # matmul chain behind the last DMA completion.
                    nc.tensor.matmul(ps_b[:, :], w2, t[:, q, :, 512:LV],
                                     start=st, stop=lastp)

            out_sb = cpool.tile([2, 1024], f32, tag="outsb")
            # ps_a stops one matmul before ps_b, so its copy overlaps
            # ps_b's final matmul; only the ps_b copy is exposed.
            nc.vector.tensor_copy(out_sb[:, 0:512], ps_a[:, :])
            nc.vector.tensor_copy(out_sb[:, 512:1024], ps_b[:, :])
            nc.sync.dma_start(out[:, :], out_sb[:, :])
    nc.compile()
    return nc


def make_weights():
    wts = np.zeros((P, 2), ml_dtypes.bfloat16)
    wts[:72, 0] = 1.0  # partitions 0:72 = rows 0:576 -> self row
    wts[72:, 1] = 1.0  # partitions 72:128 = rows 576:1024 -> cross row
    return wts


def device_partial_sums(aw, trace=False):
    """Run the 8-core SPMD reduction. aw: [B, H, S, S] f32.

    Returns (S_self_sum, S_cross_sum) each [B, LV] (unnormalized column
    sums over heads x rows), plus the BassKernelResults."""
    global _NC_CACHE
    if _NC_CACHE is None:
        _NC_CACHE = build_nc()
    nc = _NC_CACHE
    wts = make_weights()
    in_maps = []
    for c in range(N_CORES):
        b, h0 = divmod(c, 4)
        in_maps.append(
            {"aw": aw[b, h0 * 8:h0 * 8 + 8, :, :LV].astype(ml_dtypes.bfloat16),
             "wts": wts})
    res = run_bass_kernel_spmd(nc, in_maps, core_ids=list(range(N_CORES)),
                               trace=trace)
    parts = np.stack([res.results[c]["out"] for c in range(N_CORES)])
    # cols 0:512 directly; cols 512:1024 hold the eight 64-wide chunk
    # tails side by side -> sum the 8 groups to get cols 512:576.
    def unpack(rows):  # rows: [4, 1024] for one batch/score row
        head = rows[:, 0:512].sum(axis=0)
        tail = rows[:, 512:1024].reshape(-1, 8, 64).sum(axis=(0, 1))
        return np.concatenate([head, tail])
    self_sum = np.stack([unpack(parts[4 * b:4 * b + 4, 0])
                         for b in range(B)]).astype(np.float32)
    cross_sum = np.stack([unpack(parts[4 * b:4 * b + 4, 1])
                          for b in range(B)]).astype(np.float32)
    return self_sum, cross_sum, res


def _sigmoid(x):
    x = np.asarray(x, np.float32)
    out = np.empty_like(x)
    pos = x >= 0
    out[pos] = 1.0 / (1.0 + np.exp(-x[pos]))
    ex = np.exp(x[~pos])
    out[~pos] = ex / (1.0 + ex)
    return out


def _adaptive_pool(x, out_size):
    # matches torch.nn.AdaptiveAvgPool1d over the last axis
    L = x.shape[-1]
    i = np.arange(out_size)
    starts = (i * L) // out_size
    ends = ((i + 1) * L + out_size - 1) // out_size
    zeros = np.zeros(x.shape[:-1] + (1,), x.dtype)
    cs = np.concatenate([zeros, np.cumsum(x, axis=-1, dtype=np.float32)],
                        axis=-1)
    lengths = (ends - starts).astype(np.float32)
    return ((cs[..., ends] - cs[..., starts]) / lengths).astype(np.float32)


def postprocess(S_self, S_cross, W1, b1, Wr, br, Ws, bs, lv=LV):
    LAMBDA_SAMPLE = np.float32(3.0)
    TEMPERATURE = np.float32(100.0)
    Bn = S_self.shape[0]

    S_red = ((S_self + S_cross) * np.float32(0.5)).astype(np.float32)

    grid = int(lv ** 0.5)
    stride = 2
    num_sampled = (grid // stride) ** 2
    R_s = num_sampled / lv
    ii, jj = np.meshgrid(np.arange(0, grid, stride),
                         np.arange(0, grid, stride), indexing="ij")
    sampled_idx = (ii * grid + jj).reshape(-1)
    smask = np.zeros((lv,), bool)
    smask[sampled_idx] = True
    S_spat = np.where(smask[None, :],
                      np.float32(1.0 - R_s * LAMBDA_SAMPLE),
                      np.float32(-100.0)).astype(np.float32)
    S_spat = np.broadcast_to(S_spat, (Bn, lv))

    pooled = np.concatenate([_adaptive_pool(S_self, 256),
                             _adaptive_pool(S_cross, 256)], axis=-1)
    shared = np.maximum(pooled @ W1 + b1, np.float32(0.0)).astype(np.float32)
    theta_r = _sigmoid(shared @ Wr + br)
    theta_s = _sigmoid(shared @ Ws + bs)

    mask_r = _sigmoid((S_red - theta_r) * TEMPERATURE)
    mask_s = _sigmoid((S_spat - theta_s) * TEMPERATURE)
    mask = np.maximum(mask_r, mask_s)
    return mask_r, mask_s, mask


def kernel(hidden_states, attention_weights, position_ids, num_vision_tokens,
           W1, b1, Wr, br, Ws, bs):
    hs = np.asarray(hidden_states)
    aw = np.asarray(attention_weights, np.float32)
    pos = np.asarray(position_ids)
    Lv = int(num_vision_tokens)
    W1 = np.asarray(W1, np.float32)
    b1 = np.asarray(b1, np.float32)
    Wr = np.asarray(Wr, np.float32)
    br = np.asarray(br, np.float32)
    Ws = np.asarray(Ws, np.float32)
    bs = np.asarray(bs, np.float32)

    if Lv == LV and aw.shape == (B, N_HEADS, S, S):
        self_sum, cross_sum, _ = device_partial_sums(aw)
        S_self = (self_sum / np.float32(N_HEADS * LV)).astype(np.float32)
        S_cross = (cross_sum / np.float32(N_HEADS * (S - LV))).astype(
            np.float32)
    else:
        # generic fallback for unexpected shapes: full reduction on host
        S_self = aw[:, :, :Lv, :Lv].mean(axis=(1, 2), dtype=np.float32)
        S_cross = aw[:, :, Lv:, :Lv].mean(axis=(1, 2), dtype=np.float32)

    mask_r, mask_s, mask = postprocess(S_self, S_cross, W1, b1, Wr, br,
                                       Ws, bs, lv=Lv)
    return hs, pos, mask_r, mask_s, mask


# revision 32
# speedup vs baseline: 1.0977x; 1.0259x over previous
"""Trainium2 Bass kernel for nn_ATPModule_38062000177838 (topk_masking).

The only heavy compute in the module is the pair of mean-reductions over
attention_weights[:, :, :576, :576] (S_self) and [:, :, 576:, :576]
(S_cross).  hidden_states / position_ids pass through unchanged, and the
pooling + tiny MLP + sigmoid masks are O(B*576) host-side work.

Sharding: 64 (batch, head) pairs -> 8 per core (cores 0-3: batch 0,
cores 4-7: batch 1).  Each core reduces its 8 pairs to partial column
sums (row 0: self rows 0:576, row 1: cross rows 576:1024); the host
combines partials, divides by the counts, and runs the tiny MLP + masks
in numpy float32.

Device kernel design (per core):
- The host packs each pair's [1024, 576] slice to bf16 (halves HBM
  traffic; the 18k-element mean washes out quantization noise, measured
  mask error ~1e-3) in a layout where SBUF partition p holds rows
  8p..8p+7 -- each partition's tile line is one contiguous 9KB DRAM run,
  so the DMA sustains ~fabric-rate.
- Self rows (0:576) are exactly partitions 0:72, so a single [128, 2]
  0/1 weight matrix routes self/cross sums into PSUM rows 0/1 via
  TensorEngine matmuls (bf16, 1 cycle/column), 9 N=512 matmuls per pair
  (the eight 64-col chunk tails are covered by one strided matmul into a
  second PSUM bank and summed on host).
- All aw DMAs go on the strict-FIFO sync HWDGE ring in consumption
  order; the first/last pair loads are split so the PE starts ~1us after
  the first piece lands and trails the stream tail.  A few dummy matmuls
  during the DMA lead-in warm the PE out of its cold HAM clock state.
"""

import ml_dtypes
import numpy as np

import concourse.bacc as bacc
import concourse.mybir as mybir
import concourse.tile as tile
from concourse.bass_utils import run_bass_kernel_spmd

P = 128          # SBUF partitions
LV = 576         # num vision tokens
S = 1024         # sequence length
NPAIRS = 8       # (batch, head) pairs per core
NCHUNK = 8       # 1024 rows / 128 partitions
N_CORES = 8
N_HEADS = 32
B = 2

_NC_CACHE = None

# tuned on hardware; see bench.py
BUFS = 6
FIRST_SPLIT = ((0, 2), (2, 8))
LAST_SPLIT = ((0, 4), (4, 7), (7, 8))
WARM_MMS = 2
GROUP = 1


def _make_bacc(skip_const_memsets, skip_init_barrier=False):
    """Construct the Bacc assembler.  With skip_const_memsets, suppress the
    four const-AP SBUF memsets Bass.__init__ emits on GpSimd: nothing in
    this kernel reads the const APs, and they sit on the preamble critical
    path before the first DMA trigger.  With skip_init_barrier, also drop
    the all-engine barrier Bass.__init__ emits after them -- with the
    memsets gone it orders nothing (all cross-engine deps in the body are
    semaphore-managed by Tile)."""
    if not skip_const_memsets:
        return bacc.Bacc("TRN2", target_bir_lowering=False)
    import concourse.bass as bass
    cls = bass.BassGpSimd
    orig = cls.memset

    def memset_patch(self, ap, constant):
        name = getattr(getattr(ap, "tensor", None), "name", "")
        if isinstance(name, str) and name.startswith("const-"):
            return None
        return orig(self, ap, constant)

    cls.memset = memset_patch
    orig_barrier = bass.Bass.all_engine_barrier
    if skip_init_barrier:
        bass.Bass.all_engine_barrier = lambda self, **kw: None
    try:
        return bacc.Bacc("TRN2", target_bir_lowering=False)
    finally:
        cls.memset = orig
        bass.Bass.all_engine_barrier = orig_barrier


def build_nc(bufs=None, first_split=None, last_split=None, warm_mms=None,
             group=None, wsb_warm=False, skip_const=True,
             skip_barrier=True):
    bufs = BUFS if bufs is None else bufs
    first_split = FIRST_SPLIT if first_split is None else first_split
    last_split = LAST_SPLIT if last_split is None else last_split
    warm_mms = WARM_MMS if warm_mms is None else warm_mms
    group = GROUP if group is None else group
    f32 = mybir.dt.float32
    nc = _make_bacc(skip_const, skip_barrier)
    bf16 = mybir.dt.bfloat16
    # aw is host-packed to [pairs, S, LV] bf16: halves the HBM traffic
    # (the 18k-element mean reduction washes out bf16 quantization noise)
    # and the kernel reads contiguous 9KB-per-partition runs.
    aw = nc.dram_tensor("aw", [NPAIRS, S, LV], bf16, kind="ExternalInput")
    wts = nc.dram_tensor("wts", [P, 2], bf16, kind="ExternalInput")
    out = nc.dram_tensor("out", [2, 1024], f32, kind="ExternalOutput")

    with tile.TileContext(nc) as tc:
        with tc.tile_pool(name="io", bufs=bufs) as pool, \
             tc.tile_pool(name="consts", bufs=1) as cpool, \
             tc.tile_pool(name="psum", bufs=1, space="PSUM") as ppool:
            w_sb = cpool.tile([P, 2], bf16, tag="wsb")
            nc.scalar.dma_start(w_sb[:, :], wts[:, :])
            # Layout: partition p holds rows 8p..8p+7 (chunk n = row 8p+n).
            # Self rows (0:576) are exactly partitions 0:72, so one weight
            # matrix [self_mask, cross_mask] serves every matmul.
            w2 = w_sb[:, 0:2]

            # ps_a row 0/1 = self/cross sums for columns 0:512.
            # ps_b accumulates the eight 64-wide column tails (cols 512:576
            # of each chunk) side by side; the host sums the 8 groups.
            ps_a = ppool.tile([2, 512], f32, tag="psa")
            ps_b = ppool.tile([2, 512], f32, tag="psb")

            # Dummy matmuls fill the PE's cold HAM window (~3.4us at
            # 1.2GHz) during the DMA lead-in so real matmuls run at
            # 2.4GHz; also consumes the weights-DMA wait.
            ps_w = ppool.tile([2, 512], f32, tag="psw")
            dummy = cpool.tile([P, 512], bf16, tag="dummy")
            nc.vector.memset(dummy[:, :], 1.0)
            for _ in range(warm_mms):
                nc.tensor.matmul(ps_w[:, :], dummy[:, 0:2], dummy[:, :],
                                 start=True, stop=True)
            if wsb_warm:
                # Consume the weights-DMA wait before the first real matmul
                # (which may then carry only the tile-DMA wait).
                nc.tensor.matmul(ps_w[:, :], w_sb[:, 0:2], dummy[:, :],
                                 start=True, stop=True)

            # Full reduction on the TensorEngine in bf16 (1 cycle/column),
            # accumulating into f32 PSUM across all pairs.  `group` pairs
            # share one DMA (fewer, larger transfers).
            for g0 in range(0, NPAIRS, group):
                t = pool.tile([P, group, NCHUNK, LV], bf16, tag="awt")
                src = aw[g0:g0 + group].rearrange("q (p n) m -> p q n m",
                                                  n=NCHUNK)
                first = g0 == 0
                lastg = g0 + group == NPAIRS
                if first and group == 1 and first_split:
                    # Split the first load so the PE starts ~1us after the
                    # first piece lands instead of waiting for the full
                    # tile.  All aw DMAs stay on the sync ring: it is
                    # strict-FIFO, which keeps completion order equal to PE
                    # consumption order.
                    for c0, c1 in first_split:
                        nc.sync.dma_start(t[:, 0, c0:c1, :],
                                          src[:, 0, c0:c1, :])
                elif lastg and group == 1 and last_split:
                    # Split the last load too: the PE then trails the tail
                    # of the stream chunk by chunk instead of waiting for
                    # the whole tile, pulling the final matmul earlier.
                    for c0, c1 in last_split:
                        nc.sync.dma_start(t[:, 0, c0:c1, :],
                                          src[:, 0, c0:c1, :])
                else:
                    nc.sync.dma_start(t[:, :, :, :], src)
                for q in range(group):
                    st = first and q == 0
                    lastp = lastg and q == group - 1
                    for n in range(NCHUNK):
                        nc.tensor.matmul(ps_a[:, :], w2, t[:, q, n, 0:512],
                                         start=st and n == 0,
                                         stop=lastp and n == NCHUNK - 1)
                    # One strided matmul covers all eight 64-col chunk
                    # tails.  It reads the whole tile, so it runs LAST:
                    # putting it first would queue the final pair's entire
                    # matmul chain behind the last DMA completion.
                    nc.tensor.matmul(ps_b[:, :], w2, t[:, q, :, 512:LV],
                                     start=st, stop=lastp)

            out_sb = cpool.tile([2, 1024], f32, tag="outsb")
            # ps_a stops one matmul before ps_b, so its copy overlaps
            # ps_b's final matmul; only the ps_b copy is exposed.
            nc.vector.tensor_copy(out_sb[:, 0:512], ps_a[:, :])
            nc.vector.tensor_copy(out_sb[:, 512:1024], ps_b[:, :])
            nc.sync.dma_start(out[:, :], out_sb[:, :])
    nc.compile()
    return nc


def make_weights():
    wts = np.zeros((P, 2), ml_dtypes.bfloat16)
    wts[:72, 0] = 1.0  # partitions 0:72 = rows 0:576 -> self row
    wts[72:, 1] = 1.0  # partitions 72:128 = rows 576:1024 -> cross row
    return wts


def device_partial_sums(aw, trace=False):
    """Run the 8-core SPMD reduction. aw: [B, H, S, S] f32.

    Returns (S_self_sum, S_cross_sum) each [B, LV] (unnormalized column
    sums over heads x rows), plus the BassKernelResults."""
    global _NC_CACHE
    if _NC_CACHE is None:
        _NC_CACHE = build_nc()
    nc = _NC_CACHE
    wts = make_weights()
    in_maps = []
    for c in range(N_CORES):
        b, h0 = divmod(c, 4)
        in_maps.append(
            {"aw": aw[b, h0 * 8:h0 * 8 + 8, :, :LV].astype(ml_dtypes.bfloat16),
             "wts": wts})
    res = run_bass_kernel_spmd(nc, in_maps, core_ids=list(range(N_CORES)),
                               trace=trace)
    parts = np.stack([res.results[c]["out"] for c in range(N_CORES)])
    # cols 0:512 directly; cols 512:1024 hold the eight 64-wide chunk
    # tails side by side -> sum the 8 groups to get cols 512:576.
    def unpack(rows):  # rows: [4, 1024] for one batch/score row
        head = rows[:, 0:512].sum(axis=0)
        tail = rows[:, 512:1024].reshape(-1, 8, 64).sum(axis=(0, 1))
        return np.concatenate([head, tail])
    self_sum = np.stack([unpack(parts[4 * b:4 * b + 4, 0])
                         for b in range(B)]).astype(np.float32)
    cross_sum = np.stack([unpack(parts[4 * b:4 * b + 4, 1])
                          for b in range(B)]).astype(np.float32)
    return self_sum, cross_sum, res


def _sigmoid(x):
    x = np.asarray(x, np.float32)
    out = np.empty_like(x)
    pos = x >= 0
    out[pos] = 1.0 / (1.0 + np.exp(-x[pos]))
    ex = np.exp(x[~pos])
    out[~pos] = ex / (1.0 + ex)
    return out


def _adaptive_pool(x, out_size):
    # matches torch.nn.AdaptiveAvgPool1d over the last axis
    L = x.shape[-1]
    i = np.arange(out_size)
    starts = (i * L) // out_size
    ends = ((i + 1) * L + out_size - 1) // out_size
    zeros = np.zeros(x.shape[:-1] + (1,), x.dtype)
    cs = np.concatenate([zeros, np.cumsum(x, axis=-1, dtype=np.float32)],
                        axis=-1)
    lengths = (ends - starts).astype(np.float32)
    return ((cs[..., ends] - cs[..., starts]) / lengths).astype(np.float32)


def postprocess(S_self, S_cross, W1, b1, Wr, br, Ws, bs, lv=LV):
    LAMBDA_SAMPLE = np.float32(3.0)
    TEMPERATURE = np.float32(100.0)
    Bn = S_self.shape[0]

    S_red = ((S_self + S_cross) * np.float32(0.5)).astype(np.float32)

    grid = int(lv ** 0.5)
    stride = 2
    num_sampled = (grid // stride) ** 2
    R_s = num_sampled / lv
    ii, jj = np.meshgrid(np.arange(0, grid, stride),
                         np.arange(0, grid, stride), indexing="ij")
    sampled_idx = (ii * grid + jj).reshape(-1)
    smask = np.zeros((lv,), bool)
    smask[sampled_idx] = True
    S_spat = np.where(smask[None, :],
                      np.float32(1.0 - R_s * LAMBDA_SAMPLE),
                      np.float32(-100.0)).astype(np.float32)
    S_spat = np.broadcast_to(S_spat, (Bn, lv))

    pooled = np.concatenate([_adaptive_pool(S_self, 256),
                             _adaptive_pool(S_cross, 256)], axis=-1)
    shared = np.maximum(pooled @ W1 + b1, np.float32(0.0)).astype(np.float32)
    theta_r = _sigmoid(shared @ Wr + br)
    theta_s = _sigmoid(shared @ Ws + bs)

    mask_r = _sigmoid((S_red - theta_r) * TEMPERATURE)
    mask_s = _sigmoid((S_spat - theta_s) * TEMPERATURE)
    mask = np.maximum(mask_r, mask_s)
    return mask_r, mask_s, mask


def kernel(hidden_states, attention_weights, position_ids, num_vision_tokens,
           W1, b1, Wr, br, Ws, bs):
    hs = np.asarray(hidden_states)
    aw = np.asarray(attention_weights, np.float32)
    pos = np.asarray(position_ids)
    Lv = int(num_vision_tokens)
    W1 = np.asarray(W1, np.float32)
    b1 = np.asarray(b1, np.float32)
    Wr = np.asarray(Wr, np.float32)
    br = np.asarray(br, np.float32)
    Ws = np.asarray(Ws, np.float32)
    bs = np.asarray(bs, np.float32)

    if Lv == LV and aw.shape == (B, N_HEADS, S, S):
        self_sum, cross_sum, _ = device_partial_sums(aw)
        S_self = (self_sum / np.float32(N_HEADS * LV)).astype(np.float32)
        S_cross = (cross_sum / np.float32(N_HEADS * (S - LV))).astype(
            np.float32)
    else:
        # generic fallback for unexpected shapes: full reduction on host
        S_self = aw[:, :, :Lv, :Lv].mean(axis=(1, 2), dtype=np.float32)
        S_cross = aw[:, :, Lv:, :Lv].mean(axis=(1, 2), dtype=np.float32)

    mask_r, mask_s, mask = postprocess(S_self, S_cross, W1, b1, Wr, br,
                                       Ws, bs, lv=Lv)
    return hs, pos, mask_r, mask_s, mask
